# revision 44
# baseline (speedup 1.0000x reference)
"""MaxSim ranker kernel for 8 Trainium2 NeuronCores.

Primary (sparse) strategy: the host computes each batch's candidate doc set
(unique(emb2pid[token_ids])), assigns candidate docs round-robin to the 8
cores (balanced custom shard boundaries, as the sharding hint allows), and
packs per-(core, query-half) compacted fp16 V^T layouts -- each half (4
batches, 128 queries) only ever scores the ~350 docs its batches actually
reference, cutting PE/reduce/DMA work ~1.8x vs dense. Each core:
  1. streams its two compacted V^T halves through the PE (single-pass fp16
     matmul, 128 queries x ~45k doc-token columns per half),
  2. max-reduces each doc's 128 token columns, split across two pipelines:
     DVE tensor_reduce straight from PSUM (route A), and ACT-copy->SBUF fp16
     + DVE tensor_tensor max tree in its 2x 16-bit mode (route B),
  3. sums each batch's 32 queries with a block-ones matmul, adds the host
     candidate mask (half 0's tail overlaps half 1's stream),
  4. full-sorts each 32-doc chunk via max8/max_index/match_replace ->
     packed topvi output.
Host merges the 8 local lists into the global top-k via the sel lists.

A dense doc-range-sharded variant (_build_program, PASSES=1 fast / PASSES=3
fp16-hi/lo exact) is kept as a fallback for inputs whose candidate sets
overflow S_SLOTS per core-half.
"""

import sys

for _p in ("/opt/trn_rl_repo", "/root/.axon_site/_ro/trn_rl_repo"):
    if _p not in sys.path:
        sys.path.append(_p)

import numpy as np

# ---- problem constants (hardcoded per contract) ----
N_DOCS = 5000
DOC_LEN = 128
DIM = 128
B = 8
NQ = 32
NTOK = 1024
N_EMB = N_DOCS * DOC_LEN
NCORES = 8
SHARD = N_DOCS // NCORES          # 625 docs per core
SH_DOCS = 640                      # padded shard docs
COLS = SH_DOCS * DOC_LEN           # 81920 V^T columns per core
GROUP = 1024                       # one PSUM tile: 8 docs x 128 tokens
SUPER = 4                          # PSUM tiles per reduce unit (32 docs)
NCHUNK = COLS // (GROUP * SUPER)   # 20 V chunks of 4096 cols
CHUNK = 40                         # per-(row,chunk) doc count for topk export
NEG = -1.0e30

PASSES = 1                         # 1 = fp16 single pass; 3 = fp16 hi/lo exact

# ---- sparse (candidate-compacted) path ----
S_SLOTS = 384                      # doc slots per query-half per core
CHUNK2 = 32                        # docs per topk partition row (12 chunks)
NCHUNK2 = S_SLOTS * DOC_LEN // (GROUP * SUPER)  # 12 V chunks per half
TOP_EXPORT = 16                    # top docs exported per 32-doc chunk row:
                                   # P(>16 of a batch's global top-100 land in
                                   # one 32-doc chunk) is ~1e-15

_PROGRAMS = {}


def _routing(n_units, n_a, n_b, n_c):
    """Interleave A/B/C unit assignments evenly across the stream."""
    assert n_a + n_b + n_c == n_units
    counts = {"A": n_a, "B": n_b, "C": n_c}
    acc = {"A": 0.0, "B": 0.0, "C": 0.0}
    out = []
    for _ in range(n_units):
        for k in acc:
            acc[k] += counts[k] / n_units
        pick = max(acc, key=lambda k: acc[k])
        acc[pick] -= 1.0
        out.append(pick)
    # start with an A unit so the DVE has PSUM-direct work while the first
    # ACT copies are still in flight
    if "A" in out and out[0] != "A":
        i = out.index("A")
        out[0], out[i] = out[i], out[0]
    return out


def _build_program(passes=PASSES, n_a=None, n_b=None, n_c=None):
    import concourse.bass as bass
    import concourse.mybir as mybir
    import concourse.tile as tile
    from concourse import bacc

    f16 = mybir.dt.float16
    f32 = mybir.dt.float32

    nc = bacc.Bacc("TRN2", target_bir_lowering=False, debug=False)

    N_UNITS = NCHUNK * 2  # (chunk, qc) pairs of SUPER groups each
    if n_a is None:
        # defaults tuned from the engine cost model (C = gpsimd tree is
        # disabled: walrus rejects TensorTensor on Pool in this toolchain)
        if passes == 1:
            n_a, n_b, n_c = 9, 31, 0
        else:
            n_a, n_b, n_c = 6, 34, 0
    routes = _routing(N_UNITS, n_a, n_b, n_c)

    qw = nc.dram_tensor("qw", [DIM, 256 * (2 if passes == 3 else 1)], f16,
                        kind="ExternalInput")
    vh = nc.dram_tensor("vh", [DIM, COLS], f16, kind="ExternalInput")
    if passes == 3:
        vl = nc.dram_tensor("vl", [DIM, COLS], f16, kind="ExternalInput")
    maskd = nc.dram_tensor("maskd", [B, SH_DOCS], f32, kind="ExternalInput")
    # packed output: [:, :CHUNK] = scores f32, [:, CHUNK:] = indices (u32 bits)
    topvi = nc.dram_tensor("topvi", [128, 2 * CHUNK], f32, kind="ExternalOutput")

    # reduce result dtype: fp16 in fast mode (feeds fp16 ones-matmul), fp32 in
    # exact mode.
    rdt = f16 if passes == 1 else f32

    with tile.TileContext(nc) as tc:
        with (
            tc.tile_pool(name="const", bufs=1) as cpool,
            tc.tile_pool(name="v", bufs=3) as vpool,
            tc.tile_pool(name="ps", bufs=2, space="PSUM") as pspool,
            tc.tile_pool(name="stg", bufs=2) as spool,
            tc.tile_pool(name="tree", bufs=2) as tpool,
            tc.tile_pool(name="res", bufs=1) as rpool,
        ):
            qw_sb = cpool.tile([DIM, 256 * (2 if passes == 3 else 1)], f16)
            nc.sync.dma_start(qw_sb[:], qw[:])
            mask_sb = cpool.tile([B, SH_DOCS], f32)
            nc.sync.dma_start(mask_sb[:], maskd[:])

            # ones weights for the q-sum matmul: col j sums batch j's 32 queries
            ones0 = cpool.tile([DIM, 8], rdt)
            ones1 = cpool.tile([DIM, 8], rdt)
            nc.vector.memset(ones0[:], 0.0)
            nc.vector.memset(ones1[:], 0.0)
            for j in range(4):
                nc.vector.memset(ones0[32 * j : 32 * j + 32, j : j + 1], 1.0)
                nc.vector.memset(ones1[32 * j : 32 * j + 32, 4 + j : 5 + j], 1.0)

            maxres0 = rpool.tile([128, SH_DOCS], rdt)
            maxres1 = rpool.tile([128, SH_DOCS], rdt)
            maxres = (maxres0, maxres1)

            CH = GROUP * SUPER  # 4096 cols per chunk
            unit = 0
            for gc in range(NCHUNK):
                vh_t = vpool.tile([DIM, CH], f16, tag="vh")
                nc.sync.dma_start(vh_t[:], vh[:, gc * CH : (gc + 1) * CH])
                if passes == 3:
                    vl_t = vpool.tile([DIM, CH], f16, tag="vl")
                    nc.sync.dma_start(vl_t[:], vl[:, gc * CH : (gc + 1) * CH])
                for qc in range(2):
                    route = routes[unit]
                    unit += 1
                    qh = qw_sb[:, 128 * qc : 128 * qc + 128]
                    if passes == 3:
                        ql = qw_sb[:, 256 + 128 * qc : 256 + 128 * qc + 128]
                    doff = gc * (CH // DOC_LEN)  # first doc of this chunk
                    if route != "A":
                        stg = spool.tile([128, CH], rdt, tag=f"stg{qc}")
                    ps_list = []
                    for gi in range(SUPER):
                        ps = pspool.tile([128, GROUP], f32, tag=f"ps{qc}")
                        ps_list.append(ps)
                        for s in range(GROUP // 512):
                            sl = slice(512 * s, 512 * (s + 1))
                            gsl = slice(gi * GROUP + 512 * s,
                                        gi * GROUP + 512 * (s + 1))
                            nc.tensor.matmul(ps[:, sl], qh, vh_t[:, gsl],
                                             start=True, stop=(passes == 1))
                            if passes == 3:
                                nc.tensor.matmul(ps[:, sl], ql, vh_t[:, gsl],
                                                 start=False, stop=False)
                                nc.tensor.matmul(ps[:, sl], qh, vl_t[:, gsl],
                                                 start=False, stop=True)
                        gdocs = GROUP // DOC_LEN
                        if route == "A":
                            nc.vector.tensor_reduce(
                                out=maxres[qc][:, doff + gi * gdocs :
                                               doff + (gi + 1) * gdocs],
                                in_=ps[:].rearrange("p (d t) -> p d t", t=DOC_LEN),
                                axis=mybir.AxisListType.X,
                                op=mybir.AluOpType.max,
                            )
                        else:
                            # ACT converts PSUM fp32 -> SBUF fp16
                            nc.scalar.copy(
                                out=stg[:, gi * GROUP : (gi + 1) * GROUP], in_=ps[:]
                            )
                    if route != "A":
                        # 7-level pairwise max tree over tokens: [128, 32, 128]
                        # -> [128, 32]; DVE gets the 2x 16-bit mode, GPSIMD is
                        # the overflow lane.
                        eng = nc.vector if route == "B" else nc.gpsimd
                        src = stg[:].rearrange("p (d t) -> p d t", t=DOC_LEN)
                        width = DOC_LEN
                        lvl = 0
                        while width > 1:
                            half = width // 2
                            if half == 1:
                                dst = maxres[qc][:, doff : doff + 32]
                                dst_v = dst.rearrange("p (d t) -> p d t", t=1)
                            else:
                                t_t = tpool.tile([128, 32 * half], rdt,
                                                 tag=f"t{route}{qc}l{lvl}")
                                dst_v = t_t[:].rearrange("p (d t) -> p d t", t=half)
                            eng.tensor_tensor(
                                out=dst_v,
                                in0=src[:, :, 0:half],
                                in1=src[:, :, half:width],
                                op=mybir.AluOpType.max,
                            )
                            src = dst_v
                            width = half
                            lvl += 1

            # ---- q-sum (block-ones matmul), mask, topk ----
            ps_s = pspool.tile([B, SH_DOCS], f32, tag="ps0")
            for lo in range(0, SH_DOCS, 512):
                hi = min(lo + 512, SH_DOCS)
                nc.tensor.matmul(ps_s[:, lo:hi], ones0[:, :B], maxres0[:, lo:hi],
                                 start=True, stop=False)
                nc.tensor.matmul(ps_s[:, lo:hi], ones1[:, :B], maxres1[:, lo:hi],
                                 start=False, stop=True)

            work8 = rpool.tile([B, SH_DOCS], f32)
            nc.vector.tensor_add(out=work8[:], in0=ps_s[:], in1=mask_sb[:])

            # reshape [8, 640] -> [128, 40]: row b chunk c (40 docs) at
            # partition 16b+c; per-chunk top-40 extraction is then complete.
            work = rpool.tile([128, CHUNK], f32)
            nc.sync.dma_start(
                work[:],
                work8[:].rearrange("b (c d) -> b c d", d=CHUNK),
            )

            tvi = rpool.tile([128, 2 * CHUNK], f32)
            tv = tvi[:, :CHUNK]
            ti = tvi[:, CHUNK:].bitcast(mybir.dt.uint32)
            for r in range(CHUNK // 8):
                sl = slice(8 * r, 8 * r + 8)
                nc.vector.max(out=tv[:, sl], in_=work[:])
                nc.vector.max_index(out=ti[:, sl], in_max=tv[:, sl], in_values=work[:])
                nc.vector.match_replace(
                    out=work[:], in_to_replace=tv[:, sl], in_values=work[:],
                    imm_value=NEG
                )
            nc.sync.dma_start(topvi[:], tvi[:])

    nc.compile()
    return nc


def _build_sparse_program(n_a=None, n_b=None, n_active=NCHUNK2):
    """Candidate-compacted kernel: each query-half (4 batches, 128 queries)
    only scores the doc columns the host packed for it, so the PE and the
    max-reduce skip the ~45% of (half, doc) pairs no batch needs. Only
    `n_active` 32-doc chunks per half are streamed/scored."""
    import concourse.bass as bass
    import concourse.mybir as mybir
    import concourse.tile as tile
    from concourse import bacc

    f16 = mybir.dt.float16
    f32 = mybir.dt.float32

    nc = bacc.Bacc("TRN2", target_bir_lowering=False, debug=False)

    N_UNITS = 2 * n_active  # units of SUPER groups (32 docs each)
    if n_a is None:
        n_a = max(2, round(N_UNITS * 0.23))
        n_b = N_UNITS - n_a
    routes = _routing(N_UNITS, n_a, n_b, N_UNITS - n_a - n_b)

    CH = GROUP * SUPER  # 4096
    HC = n_active * CH  # cols per half
    SA = n_active * CHUNK2  # active doc slots per half
    qw = nc.dram_tensor("qw", [DIM, 256], f16, kind="ExternalInput")
    vh = nc.dram_tensor("vh", [DIM, 2 * HC], f16, kind="ExternalInput")
    # mask rows = relative batch (0-3), halves side by side in the free dim
    maskd = nc.dram_tensor("maskd", [4, 2 * S_SLOTS], f32, kind="ExternalInput")
    NP = 2 * 4 * n_active  # topk partitions: (half, batch, chunk)
    topvi = nc.dram_tensor("topvi", [NP, 2 * TOP_EXPORT], f32,
                           kind="ExternalOutput")

    with tile.TileContext(nc) as tc:
        with (
            tc.tile_pool(name="const", bufs=1) as cpool,
            tc.tile_pool(name="v", bufs=4) as vpool,
            tc.tile_pool(name="ps", bufs=2, space="PSUM") as pspool,
            tc.tile_pool(name="stg", bufs=2) as spool,
            tc.tile_pool(name="tree", bufs=2) as tpool,
            tc.tile_pool(name="res", bufs=1) as rpool,
        ):
            qw_sb = cpool.tile([DIM, 256], f16)
            nc.sync.dma_start(qw_sb[:], qw[:])
            # mask is only needed at the tail; its DMA is emitted a few chunks
            # into the stream so the first V chunks aren't queued behind it
            mask_sb = cpool.tile([4, 2 * S_SLOTS], f32)

            # ones: col j sums the 32 queries of (relative) batch j
            ones = cpool.tile([DIM, 4], f16)
            nc.vector.memset(ones[:], 0.0)
            for j in range(4):
                nc.vector.memset(ones[32 * j : 32 * j + 32, j : j + 1], 1.0)

            maxres0 = rpool.tile([128, SA], f16)
            maxres1 = rpool.tile([128, SA], f16)
            maxres = (maxres0, maxres1)
            work = rpool.tile([NP, CHUNK2], f32)

            def qsum_tail(h):
                # q-sum + mask + reshape for half h (emitted once that half's
                # maxres is complete; h=0's runs during h=1's stream)
                ps_s = pspool.tile([4, SA], f32, tag="ps")
                nc.tensor.matmul(ps_s[:], ones[:, :4], maxres[h][:],
                                 start=True, stop=True)
                w8 = rpool.tile([4, SA], f32, name=f"w8_{h}")
                nc.vector.tensor_add(
                    out=w8[:], in0=ps_s[:],
                    in1=mask_sb[:, h * S_SLOTS : h * S_SLOTS + SA])
                nc.sync.dma_start(
                    work[(NP // 2) * h : (NP // 2) * (h + 1), :],
                    w8[:].rearrange("b (c d) -> b c d", d=CHUNK2),
                )

            unit = 0
            for h in range(2):
                qh = qw_sb[:, 128 * h : 128 * h + 128]
                for gc in range(n_active):
                    if h == 0 and gc == 2:
                        nc.sync.dma_start(mask_sb[:], maskd[:])
                    if h == 1 and gc == 3:
                        qsum_tail(0)
                    vh_t = vpool.tile([DIM, CH], f16, tag="vh")
                    nc.sync.dma_start(
                        vh_t[:], vh[:, h * HC + gc * CH : h * HC + (gc + 1) * CH])
                    route = routes[unit]
                    unit += 1
                    doff = gc * (CH // DOC_LEN)
                    if route != "A":
                        stg = spool.tile([128, CH], f16, tag=f"stg{h}")
                    G2 = 2048  # sparse-path PSUM tile: halves ACT/DVE
                    for gi in range(CH // G2):  # fixed per-instruction costs
                        ps = pspool.tile([128, G2], f32, tag="ps")
                        for s in range(G2 // 512):
                            sl = slice(512 * s, 512 * (s + 1))
                            gsl = slice(gi * G2 + 512 * s,
                                        gi * G2 + 512 * (s + 1))
                            nc.tensor.matmul(ps[:, sl], qh, vh_t[:, gsl],
                                             start=True, stop=True)
                        gdocs = G2 // DOC_LEN
                        if route == "A":
                            nc.vector.tensor_reduce(
                                out=maxres[h][:, doff + gi * gdocs :
                                              doff + (gi + 1) * gdocs],
                                in_=ps[:].rearrange("p (d t) -> p d t", t=DOC_LEN),
                                axis=mybir.AxisListType.X,
                                op=mybir.AluOpType.max,
                            )
                        else:
                            nc.scalar.copy(
                                out=stg[:, gi * G2 : (gi + 1) * G2], in_=ps[:])
                    if route != "A":
                        # pairwise-max tree (2x 16-bit mode) down to width 16,
                        # then one tensor_reduce: the last levels are
                        # fixed-cost dominated as separate instructions
                        src = stg[:].rearrange("p (d t) -> p d t", t=DOC_LEN)
                        width = DOC_LEN
                        lvl = 0
                        while width > 16:
                            half = width // 2
                            t_t = tpool.tile([128, 32 * half], f16,
                                             tag=f"t{h}l{lvl}")
                            dst_v = t_t[:].rearrange("p (d t) -> p d t", t=half)
                            nc.vector.tensor_tensor(
                                out=dst_v, in0=src[:, :, 0:half],
                                in1=src[:, :, half:width],
                                op=mybir.AluOpType.max,
                            )
                            src = dst_v
                            width = half
                            lvl += 1
                        nc.vector.tensor_reduce(
                            out=maxres[h][:, doff : doff + 32],
                            in_=src,
                            axis=mybir.AxisListType.X,
                            op=mybir.AluOpType.max,
                        )

            # ---- remaining tail: half-1 q-sum + topk ----
            qsum_tail(1)

            tvi = rpool.tile([NP, 2 * TOP_EXPORT], f32)
            tv = tvi[:, :TOP_EXPORT]
            ti = tvi[:, TOP_EXPORT:].bitcast(mybir.dt.uint32)
            for r in range(TOP_EXPORT // 8):
                sl = slice(8 * r, 8 * r + 8)
                nc.vector.max(out=tv[:, sl], in_=work[:])
                nc.vector.max_index(out=ti[:, sl], in_max=tv[:, sl], in_values=work[:])
                nc.vector.match_replace(
                    out=work[:], in_to_replace=tv[:, sl], in_values=work[:],
                    imm_value=NEG
                )
            nc.sync.dma_start(topvi[:], tvi[:])

    nc.compile()
    return nc


def _get_program(**kw):
    key = tuple(sorted(kw.items()))
    if key not in _PROGRAMS:
        if kw.pop("sparse", False):
            _PROGRAMS[key] = _build_sparse_program(**kw)
        else:
            _PROGRAMS[key] = _build_program(**kw)
    return _PROGRAMS[key]


def _fp16_split(x):
    hi = x.astype(np.float16)
    lo = (x - hi.astype(np.float32)).astype(np.float16)
    return hi, lo


def _prepare_in_maps(q_vectors, token_ids, vectors, emb2pid, passes=PASSES):
    q = np.ascontiguousarray(np.asarray(q_vectors, dtype=np.float32))
    V = np.asarray(vectors, dtype=np.float32)
    tok = np.asarray(token_ids).astype(np.int64)
    e2p = np.asarray(emb2pid).astype(np.int64)

    qt = np.ascontiguousarray(q.reshape(B * NQ, DIM).T)      # [128, 256]
    if passes == 3:
        qh, ql = _fp16_split(qt)
        qw_np = np.concatenate([qh, ql], axis=1)             # [128, 512]
    else:
        qw_np = qt.astype(np.float16)

    # host-side candidate masks: pids hit per batch, mapped to local doc ids
    pids = e2p[np.clip(tok, 0, N_EMB - 1)]                   # [B, NTOK]
    pids = np.where((tok < 0) | (tok >= N_EMB), -1, pids)
    pids = np.where((pids < 0) | (pids >= N_DOCS), -1, pids)

    in_maps = []
    for c in range(NCORES):
        vs = V[c * SHARD : (c + 1) * SHARD]                  # [625, 128, 128]
        vt = vs.transpose(2, 0, 1).reshape(DIM, SHARD * DOC_LEN)
        vt_p = np.zeros((DIM, COLS), np.float32)
        vt_p[:, : SHARD * DOC_LEN] = vt
        mask = np.full((B, SH_DOCS), NEG, np.float32)
        lo, hi = c * SHARD, (c + 1) * SHARD
        for b in range(B):
            local = pids[b][(pids[b] >= lo) & (pids[b] < hi)] - lo
            mask[b, local] = 0.0
        m = {"qw": qw_np, "maskd": mask}
        if passes == 3:
            m["vh"], m["vl"] = _fp16_split(vt_p)
        else:
            m["vh"] = vt_p.astype(np.float16)
        in_maps.append(m)
    return in_maps


def _prepare_sparse_in_maps(q_vectors, token_ids, vectors, emb2pid):
    """Compute per-batch candidate docs, balance them across cores, and pack
    per-(core, query-half) compacted fp16 V^T layouts + slot masks.
    Returns (in_maps, sels) or None if any core-half overflows S_SLOTS."""
    q = np.ascontiguousarray(np.asarray(q_vectors, dtype=np.float32))
    V = np.asarray(vectors, dtype=np.float32)
    tok = np.asarray(token_ids).astype(np.int64)
    e2p = np.asarray(emb2pid).astype(np.int64)

    qt = np.ascontiguousarray(q.reshape(B * NQ, DIM).T)      # [128, 256]
    qw_np = qt.astype(np.float16)

    pids = e2p[np.clip(tok, 0, N_EMB - 1)]
    pids = np.where((tok < 0) | (tok >= N_EMB), -1, pids)
    pids = np.where((pids < 0) | (pids >= N_DOCS), -1, pids)

    need = np.zeros((B, N_DOCS), bool)
    for b in range(B):
        u = pids[b][pids[b] >= 0]
        need[b, u] = True
    need_h = np.stack([need[:4].any(axis=0), need[4:].any(axis=0)])  # [2, N]

    # balanced doc -> core assignment: round-robin within each need category
    core_of = np.full(N_DOCS, -1, np.int32)
    for m in (need_h[0] & need_h[1], need_h[0] & ~need_h[1],
              ~need_h[0] & need_h[1]):
        idx = np.where(m)[0]
        core_of[idx] = np.arange(len(idx)) % NCORES

    all_sels = []
    max_count = 1
    for c in range(NCORES):
        csel = []
        for h in range(2):
            sel = np.where((core_of == c) & need_h[h])[0]
            if len(sel) > S_SLOTS:
                return None  # overflow: caller falls back to dense
            max_count = max(max_count, len(sel))
            csel.append(sel)
        all_sels.append(csel)
    n_active = -(-max_count // CHUNK2)  # active 32-doc chunks per half
    HC = n_active * CHUNK2 * DOC_LEN

    in_maps = []
    for c in range(NCORES):
        vh_np = np.zeros((DIM, 2 * HC), np.float16)
        mask = np.full((4, 2 * S_SLOTS), NEG, np.float32)
        for h in range(2):
            sel = all_sels[c][h]
            if len(sel):
                vt = V[sel].transpose(2, 0, 1).reshape(DIM, len(sel) * DOC_LEN)
                vh_np[:, h * HC : h * HC + vt.shape[1]] = vt.astype(np.float16)
            for j in range(4):
                mask[j, h * S_SLOTS : h * S_SLOTS + len(sel)] = np.where(
                    need[4 * h + j, sel], 0.0, NEG)
        in_maps.append({"qw": qw_np, "vh": vh_np, "maskd": mask})
    return in_maps, all_sels, n_active


def _merge_sparse(results, sels, n_active, k_val):
    top_scores = np.empty((B, k_val), np.float32)
    top_pids = np.empty((B, k_val), np.int32)
    all_v = [[] for _ in range(B)]
    all_i = [[] for _ in range(B)]
    hp = 4 * n_active  # partitions per half
    for c in range(NCORES):
        tvi = np.asarray(results[c]["topvi"], np.float32)  # [2*hp, 2*TOP_EXPORT]
        tv = tvi[:, :TOP_EXPORT]
        ti = tvi[:, TOP_EXPORT:].copy().view(np.uint32).astype(np.int64)
        for p in range(tvi.shape[0]):
            h, b_loc, ch = p // hp, (p % hp) // n_active, p % n_active
            b = 4 * h + b_loc
            slots = ch * CHUNK2 + ti[p]
            sel = sels[c][h]
            valid = (tv[p] > -1.0e29) & (slots < len(sel))
            all_v[b].append(tv[p][valid])
            all_i[b].append(sel[slots[valid]])
    for b in range(B):
        v = np.concatenate(all_v[b])
        i = np.concatenate(all_i[b])
        order = np.argsort(-v, kind="stable")[:k_val]
        top_scores[b] = v[order]
        top_pids[b] = i[order].astype(np.int32)
    return top_scores, top_pids


def _merge(results, k_val):
    top_scores = np.empty((B, k_val), np.float32)
    top_pids = np.empty((B, k_val), np.int32)
    nchunk = SH_DOCS // CHUNK  # 16
    all_v = [[] for _ in range(B)]
    all_i = [[] for _ in range(B)]
    for c in range(NCORES):
        tvi = np.asarray(results[c]["topvi"], np.float32)
        tv = tvi[:, :CHUNK].reshape(B, nchunk, CHUNK)
        ti = (tvi[:, CHUNK:].copy().view(np.uint32).astype(np.int64)
              .reshape(B, nchunk, CHUNK))
        base = c * SHARD + np.arange(nchunk)[:, None] * CHUNK  # [nchunk, 1]
        for b in range(B):
            all_v[b].append(tv[b].reshape(-1))
            all_i[b].append((ti[b] + base).reshape(-1))
    for b in range(B):
        v = np.concatenate(all_v[b])
        i = np.concatenate(all_i[b])
        valid = v > -1.0e29
        v = v[valid]
        i = i[valid]
        order = np.argsort(-v, kind="stable")[:k_val]
        top_scores[b] = v[order]
        top_pids[b] = i[order].astype(np.int32)
    return top_scores, top_pids


def _run(inputs, trace=False, trace_kwargs=None, program_kwargs=None):
    from concourse.bass_utils import run_bass_kernel_spmd

    pk = dict(program_kwargs or {})
    k_val = int(np.asarray(inputs.get("k", 100)))
    dense = pk.pop("dense", False)
    prep = None
    if not dense and pk.get("passes", PASSES) == 1:
        prep = _prepare_sparse_in_maps(
            inputs["q_vectors"], inputs["token_ids"], inputs["vectors"],
            inputs["emb2pid"])
    if prep is not None:
        in_maps, sels, n_active = prep
        nc = _get_program(sparse=True, n_active=n_active,
                          **{k: v for k, v in pk.items() if k in ("n_a", "n_b")})
        br = run_bass_kernel_spmd(
            nc, in_maps, list(range(NCORES)), trace=trace, **(trace_kwargs or {})
        )
        outs = _merge_sparse(br.results, sels, n_active, k_val)
        return outs, br
    nc = _get_program(**pk)
    in_maps = _prepare_in_maps(
        inputs["q_vectors"], inputs["token_ids"], inputs["vectors"],
        inputs["emb2pid"], passes=pk.get("passes", PASSES)
    )
    br = run_bass_kernel_spmd(
        nc, in_maps, list(range(NCORES)), trace=trace, **(trace_kwargs or {})
    )
    outs = _merge(br.results, k_val)
    return outs, br


def kernel(q_vectors, token_ids, vectors, emb2pid, k=100):
    outs, _ = _run(
        {
            "q_vectors": q_vectors,
            "token_ids": token_ids,
            "vectors": vectors,
            "emb2pid": emb2pid,
            "k": k,
        }
    )
    return outs


# revision 46
# speedup vs baseline: 1.1268x; 1.1268x over previous
"""MaxSim ranker kernel for 8 Trainium2 NeuronCores.

Primary (sparse) strategy: the host computes each batch's candidate doc set
(unique(emb2pid[token_ids])), assigns candidate docs round-robin to the 8
cores (balanced custom shard boundaries, as the sharding hint allows), and
packs per-(core, query-half) compacted fp16 V^T layouts -- each half (4
batches, 128 queries) only ever scores the ~350 docs its batches actually
reference, cutting PE/reduce/DMA work ~1.8x vs dense. Each core:
  1. streams its two compacted V^T halves through the PE (single-pass fp16
     matmul, 128 queries x ~45k doc-token columns per half),
  2. max-reduces each doc's 128 token columns, split across two pipelines:
     DVE tensor_reduce straight from PSUM (route A), and ACT-copy->SBUF fp16
     + DVE tensor_tensor max tree in its 2x 16-bit mode (route B),
  3. sums each batch's 32 queries with a block-ones matmul, adds the host
     candidate mask (half 0's tail overlaps half 1's stream),
  4. full-sorts each 32-doc chunk via max8/max_index/match_replace ->
     packed topvi output.
Host merges the 8 local lists into the global top-k via the sel lists.

A dense doc-range-sharded variant (_build_program, PASSES=1 fast / PASSES=3
fp16-hi/lo exact) is kept as a fallback for inputs whose candidate sets
overflow S_SLOTS per core-half.
"""

import sys

for _p in ("/opt/trn_rl_repo", "/root/.axon_site/_ro/trn_rl_repo"):
    if _p not in sys.path:
        sys.path.append(_p)

import numpy as np

# ---- problem constants (hardcoded per contract) ----
N_DOCS = 5000
DOC_LEN = 128
DIM = 128
B = 8
NQ = 32
NTOK = 1024
N_EMB = N_DOCS * DOC_LEN
NCORES = 8
SHARD = N_DOCS // NCORES          # 625 docs per core
SH_DOCS = 640                      # padded shard docs
COLS = SH_DOCS * DOC_LEN           # 81920 V^T columns per core
GROUP = 1024                       # one PSUM tile: 8 docs x 128 tokens
SUPER = 4                          # PSUM tiles per reduce unit (32 docs)
NCHUNK = COLS // (GROUP * SUPER)   # 20 V chunks of 4096 cols
CHUNK = 40                         # per-(row,chunk) doc count for topk export
NEG = -1.0e30

PASSES = 1                         # 1 = fp16 single pass; 3 = fp16 hi/lo exact

# ---- sparse (candidate-compacted) path ----
S_SLOTS = 384                      # doc slots per query-half per core
CHUNK2 = 32                        # docs per topk partition row (12 chunks)
NCHUNK2 = S_SLOTS * DOC_LEN // (GROUP * SUPER)  # 12 V chunks per half
TOP_EXPORT = 16                    # top docs exported per 32-doc chunk row:
                                   # P(>16 of a batch's global top-100 land in
                                   # one 32-doc chunk) is ~1e-15

_PROGRAMS = {}


def _routing(n_units, n_a, n_b, n_c):
    """Interleave A/B/C unit assignments evenly across the stream."""
    assert n_a + n_b + n_c == n_units
    counts = {"A": n_a, "B": n_b, "C": n_c}
    acc = {"A": 0.0, "B": 0.0, "C": 0.0}
    out = []
    for _ in range(n_units):
        for k in acc:
            acc[k] += counts[k] / n_units
        pick = max(acc, key=lambda k: acc[k])
        acc[pick] -= 1.0
        out.append(pick)
    # start with an A unit so the DVE has PSUM-direct work while the first
    # ACT copies are still in flight
    if "A" in out and out[0] != "A":
        i = out.index("A")
        out[0], out[i] = out[i], out[0]
    return out


def _build_program(passes=PASSES, n_a=None, n_b=None, n_c=None):
    import concourse.bass as bass
    import concourse.mybir as mybir
    import concourse.tile as tile
    from concourse import bacc

    f16 = mybir.dt.float16
    f32 = mybir.dt.float32

    nc = bacc.Bacc("TRN2", target_bir_lowering=False, debug=False)

    N_UNITS = NCHUNK * 2  # (chunk, qc) pairs of SUPER groups each
    if n_a is None:
        # defaults tuned from the engine cost model (C = gpsimd tree is
        # disabled: walrus rejects TensorTensor on Pool in this toolchain)
        if passes == 1:
            n_a, n_b, n_c = 9, 31, 0
        else:
            n_a, n_b, n_c = 6, 34, 0
    routes = _routing(N_UNITS, n_a, n_b, n_c)

    qw = nc.dram_tensor("qw", [DIM, 256 * (2 if passes == 3 else 1)], f16,
                        kind="ExternalInput")
    vh = nc.dram_tensor("vh", [DIM, COLS], f16, kind="ExternalInput")
    if passes == 3:
        vl = nc.dram_tensor("vl", [DIM, COLS], f16, kind="ExternalInput")
    maskd = nc.dram_tensor("maskd", [B, SH_DOCS], f32, kind="ExternalInput")
    # packed output: [:, :CHUNK] = scores f32, [:, CHUNK:] = indices (u32 bits)
    topvi = nc.dram_tensor("topvi", [128, 2 * CHUNK], f32, kind="ExternalOutput")

    # reduce result dtype: fp16 in fast mode (feeds fp16 ones-matmul), fp32 in
    # exact mode.
    rdt = f16 if passes == 1 else f32

    with tile.TileContext(nc) as tc:
        with (
            tc.tile_pool(name="const", bufs=1) as cpool,
            tc.tile_pool(name="v", bufs=3) as vpool,
            tc.tile_pool(name="ps", bufs=2, space="PSUM") as pspool,
            tc.tile_pool(name="stg", bufs=2) as spool,
            tc.tile_pool(name="tree", bufs=2) as tpool,
            tc.tile_pool(name="res", bufs=1) as rpool,
        ):
            qw_sb = cpool.tile([DIM, 256 * (2 if passes == 3 else 1)], f16)
            nc.sync.dma_start(qw_sb[:], qw[:])
            mask_sb = cpool.tile([B, SH_DOCS], f32)
            nc.sync.dma_start(mask_sb[:], maskd[:])

            # ones weights for the q-sum matmul: col j sums batch j's 32 queries
            ones0 = cpool.tile([DIM, 8], rdt)
            ones1 = cpool.tile([DIM, 8], rdt)
            nc.vector.memset(ones0[:], 0.0)
            nc.vector.memset(ones1[:], 0.0)
            for j in range(4):
                nc.vector.memset(ones0[32 * j : 32 * j + 32, j : j + 1], 1.0)
                nc.vector.memset(ones1[32 * j : 32 * j + 32, 4 + j : 5 + j], 1.0)

            maxres0 = rpool.tile([128, SH_DOCS], rdt)
            maxres1 = rpool.tile([128, SH_DOCS], rdt)
            maxres = (maxres0, maxres1)

            CH = GROUP * SUPER  # 4096 cols per chunk
            unit = 0
            for gc in range(NCHUNK):
                vh_t = vpool.tile([DIM, CH], f16, tag="vh")
                nc.sync.dma_start(vh_t[:], vh[:, gc * CH : (gc + 1) * CH])
                if passes == 3:
                    vl_t = vpool.tile([DIM, CH], f16, tag="vl")
                    nc.sync.dma_start(vl_t[:], vl[:, gc * CH : (gc + 1) * CH])
                for qc in range(2):
                    route = routes[unit]
                    unit += 1
                    qh = qw_sb[:, 128 * qc : 128 * qc + 128]
                    if passes == 3:
                        ql = qw_sb[:, 256 + 128 * qc : 256 + 128 * qc + 128]
                    doff = gc * (CH // DOC_LEN)  # first doc of this chunk
                    if route != "A":
                        stg = spool.tile([128, CH], rdt, tag=f"stg{qc}")
                    ps_list = []
                    for gi in range(SUPER):
                        ps = pspool.tile([128, GROUP], f32, tag=f"ps{qc}")
                        ps_list.append(ps)
                        for s in range(GROUP // 512):
                            sl = slice(512 * s, 512 * (s + 1))
                            gsl = slice(gi * GROUP + 512 * s,
                                        gi * GROUP + 512 * (s + 1))
                            nc.tensor.matmul(ps[:, sl], qh, vh_t[:, gsl],
                                             start=True, stop=(passes == 1))
                            if passes == 3:
                                nc.tensor.matmul(ps[:, sl], ql, vh_t[:, gsl],
                                                 start=False, stop=False)
                                nc.tensor.matmul(ps[:, sl], qh, vl_t[:, gsl],
                                                 start=False, stop=True)
                        gdocs = GROUP // DOC_LEN
                        if route == "A":
                            nc.vector.tensor_reduce(
                                out=maxres[qc][:, doff + gi * gdocs :
                                               doff + (gi + 1) * gdocs],
                                in_=ps[:].rearrange("p (d t) -> p d t", t=DOC_LEN),
                                axis=mybir.AxisListType.X,
                                op=mybir.AluOpType.max,
                            )
                        else:
                            # ACT converts PSUM fp32 -> SBUF fp16
                            nc.scalar.copy(
                                out=stg[:, gi * GROUP : (gi + 1) * GROUP], in_=ps[:]
                            )
                    if route != "A":
                        # 7-level pairwise max tree over tokens: [128, 32, 128]
                        # -> [128, 32]; DVE gets the 2x 16-bit mode, GPSIMD is
                        # the overflow lane.
                        eng = nc.vector if route == "B" else nc.gpsimd
                        src = stg[:].rearrange("p (d t) -> p d t", t=DOC_LEN)
                        width = DOC_LEN
                        lvl = 0
                        while width > 1:
                            half = width // 2
                            if half == 1:
                                dst = maxres[qc][:, doff : doff + 32]
                                dst_v = dst.rearrange("p (d t) -> p d t", t=1)
                            else:
                                t_t = tpool.tile([128, 32 * half], rdt,
                                                 tag=f"t{route}{qc}l{lvl}")
                                dst_v = t_t[:].rearrange("p (d t) -> p d t", t=half)
                            eng.tensor_tensor(
                                out=dst_v,
                                in0=src[:, :, 0:half],
                                in1=src[:, :, half:width],
                                op=mybir.AluOpType.max,
                            )
                            src = dst_v
                            width = half
                            lvl += 1

            # ---- q-sum (block-ones matmul), mask, topk ----
            ps_s = pspool.tile([B, SH_DOCS], f32, tag="ps0")
            for lo in range(0, SH_DOCS, 512):
                hi = min(lo + 512, SH_DOCS)
                nc.tensor.matmul(ps_s[:, lo:hi], ones0[:, :B], maxres0[:, lo:hi],
                                 start=True, stop=False)
                nc.tensor.matmul(ps_s[:, lo:hi], ones1[:, :B], maxres1[:, lo:hi],
                                 start=False, stop=True)

            work8 = rpool.tile([B, SH_DOCS], f32)
            nc.vector.tensor_add(out=work8[:], in0=ps_s[:], in1=mask_sb[:])

            # reshape [8, 640] -> [128, 40]: row b chunk c (40 docs) at
            # partition 16b+c; per-chunk top-40 extraction is then complete.
            work = rpool.tile([128, CHUNK], f32)
            nc.sync.dma_start(
                work[:],
                work8[:].rearrange("b (c d) -> b c d", d=CHUNK),
            )

            tvi = rpool.tile([128, 2 * CHUNK], f32)
            tv = tvi[:, :CHUNK]
            ti = tvi[:, CHUNK:].bitcast(mybir.dt.uint32)
            for r in range(CHUNK // 8):
                sl = slice(8 * r, 8 * r + 8)
                nc.vector.max(out=tv[:, sl], in_=work[:])
                nc.vector.max_index(out=ti[:, sl], in_max=tv[:, sl], in_values=work[:])
                nc.vector.match_replace(
                    out=work[:], in_to_replace=tv[:, sl], in_values=work[:],
                    imm_value=NEG
                )
            nc.sync.dma_start(topvi[:], tvi[:])

    nc.compile()
    return nc


def _build_sparse_program(n_a=None, n_b=None, n_active=NCHUNK2):
    """Candidate-compacted kernel: each query-half (4 batches, 128 queries)
    only scores the doc columns the host packed for it, so the PE and the
    max-reduce skip the ~45% of (half, doc) pairs no batch needs. Only
    `n_active` 32-doc chunks per half are streamed/scored."""
    import concourse.bass as bass
    import concourse.mybir as mybir
    import concourse.tile as tile
    from concourse import bacc

    f16 = mybir.dt.float16
    f32 = mybir.dt.float32

    nc = bacc.Bacc("TRN2", target_bir_lowering=False, debug=False)

    N_UNITS = 2 * n_active  # units of SUPER groups (32 docs each)
    if n_a is None:
        n_a = max(2, round(N_UNITS * 0.23))
        n_b = N_UNITS - n_a
    routes = _routing(N_UNITS, n_a, n_b, N_UNITS - n_a - n_b)

    CH = GROUP * SUPER  # 4096
    HC = n_active * CH  # cols per half
    SA = n_active * CHUNK2  # active doc slots per half
    qw = nc.dram_tensor("qw", [DIM, 256], f16, kind="ExternalInput")
    vh = nc.dram_tensor("vh", [DIM, 2 * HC], f16, kind="ExternalInput")
    # mask rows = relative batch (0-3), halves side by side in the free dim
    maskd = nc.dram_tensor("maskd", [4, 2 * S_SLOTS], f32, kind="ExternalInput")
    NP = 2 * 4 * n_active  # topk partitions: (half, batch, chunk)
    topvi = nc.dram_tensor("topvi", [NP, 2 * TOP_EXPORT], f32,
                           kind="ExternalOutput")

    with tile.TileContext(nc) as tc:
        with (
            tc.tile_pool(name="const", bufs=1) as cpool,
            tc.tile_pool(name="v", bufs=4) as vpool,
            tc.tile_pool(name="ps", bufs=4, space="PSUM") as pspool,
            tc.tile_pool(name="stg", bufs=2) as spool,
            tc.tile_pool(name="tree", bufs=2) as tpool,
            tc.tile_pool(name="res", bufs=1) as rpool,
        ):
            qw_sb = cpool.tile([DIM, 256], f16)
            nc.sync.dma_start(qw_sb[:], qw[:])
            # mask is only needed at the tail; its DMA is emitted a few chunks
            # into the stream so the first V chunks aren't queued behind it
            mask_sb = cpool.tile([4, 2 * S_SLOTS], f32)

            # ones: col j sums the 32 queries of (relative) batch j
            ones = cpool.tile([DIM, 4], f16)
            nc.vector.memset(ones[:], 0.0)
            for j in range(4):
                nc.vector.memset(ones[32 * j : 32 * j + 32, j : j + 1], 1.0)

            maxres0 = rpool.tile([128, SA], f16)
            maxres1 = rpool.tile([128, SA], f16)
            maxres = (maxres0, maxres1)
            work = rpool.tile([NP, CHUNK2], f32)

            # warm the PE p-state while the first V chunk is in flight: the
            # clock needs ~3us of continuous execution to leave the slow
            # p-states, and the PE would otherwise idle until V arrives.
            # Results are garbage and never read.
            warm_ps = pspool.tile([128, 256], f32, tag="ps")
            for _ in range(16):
                nc.tensor.matmul(warm_ps[:], qw_sb[:, 0:128], qw_sb[:],
                                 start=True, stop=True)

            def qsum_tail(h):
                # q-sum + mask + reshape for half h (emitted once that half's
                # maxres is complete; h=0's runs during h=1's stream)
                ps_s = pspool.tile([4, SA], f32, tag="ps")
                nc.tensor.matmul(ps_s[:], ones[:, :4], maxres[h][:],
                                 start=True, stop=True)
                w8 = rpool.tile([4, SA], f32, name=f"w8_{h}")
                nc.vector.tensor_add(
                    out=w8[:], in0=ps_s[:],
                    in1=mask_sb[:, h * S_SLOTS : h * S_SLOTS + SA])
                nc.sync.dma_start(
                    work[(NP // 2) * h : (NP // 2) * (h + 1), :],
                    w8[:].rearrange("b (c d) -> b c d", d=CHUNK2),
                )

            unit = 0
            for h in range(2):
                qh = qw_sb[:, 128 * h : 128 * h + 128]
                for gc in range(n_active):
                    if h == 0 and gc == 2:
                        nc.sync.dma_start(mask_sb[:], maskd[:])
                    if h == 1 and gc == 3:
                        qsum_tail(0)
                    vh_t = vpool.tile([DIM, CH], f16, tag="vh")
                    nc.sync.dma_start(
                        vh_t[:], vh[:, h * HC + gc * CH : h * HC + (gc + 1) * CH])
                    route = routes[unit]
                    unit += 1
                    doff = gc * (CH // DOC_LEN)
                    if route != "A":
                        stg = spool.tile([128, CH], f16, tag=f"stg{h}")
                    for gi in range(SUPER):
                        ps = pspool.tile([128, GROUP], f32, tag="ps")
                        for s in range(GROUP // 512):
                            sl = slice(512 * s, 512 * (s + 1))
                            gsl = slice(gi * GROUP + 512 * s,
                                        gi * GROUP + 512 * (s + 1))
                            nc.tensor.matmul(ps[:, sl], qh, vh_t[:, gsl],
                                             start=True, stop=True)
                        gdocs = GROUP // DOC_LEN
                        if route == "A":
                            nc.vector.tensor_reduce(
                                out=maxres[h][:, doff + gi * gdocs :
                                              doff + (gi + 1) * gdocs],
                                in_=ps[:].rearrange("p (d t) -> p d t", t=DOC_LEN),
                                axis=mybir.AxisListType.X,
                                op=mybir.AluOpType.max,
                            )
                        else:
                            nc.scalar.copy(
                                out=stg[:, gi * GROUP : (gi + 1) * GROUP], in_=ps[:])
                    if route != "A":
                        # pairwise-max tree (2x 16-bit mode) down to width 16,
                        # then one tensor_reduce: the last levels are
                        # fixed-cost dominated as separate instructions
                        src = stg[:].rearrange("p (d t) -> p d t", t=DOC_LEN)
                        width = DOC_LEN
                        lvl = 0
                        while width > 16:
                            half = width // 2
                            t_t = tpool.tile([128, 32 * half], f16,
                                             tag=f"t{h}l{lvl}")
                            dst_v = t_t[:].rearrange("p (d t) -> p d t", t=half)
                            nc.vector.tensor_tensor(
                                out=dst_v, in0=src[:, :, 0:half],
                                in1=src[:, :, half:width],
                                op=mybir.AluOpType.max,
                            )
                            src = dst_v
                            width = half
                            lvl += 1
                        nc.vector.tensor_reduce(
                            out=maxres[h][:, doff : doff + 32],
                            in_=src,
                            axis=mybir.AxisListType.X,
                            op=mybir.AluOpType.max,
                        )

            # ---- remaining tail: half-1 q-sum + topk ----
            qsum_tail(1)

            tvi = rpool.tile([NP, 2 * TOP_EXPORT], f32)
            tv = tvi[:, :TOP_EXPORT]
            ti = tvi[:, TOP_EXPORT:].bitcast(mybir.dt.uint32)
            for r in range(TOP_EXPORT // 8):
                sl = slice(8 * r, 8 * r + 8)
                nc.vector.max(out=tv[:, sl], in_=work[:])
                nc.vector.max_index(out=ti[:, sl], in_max=tv[:, sl], in_values=work[:])
                nc.vector.match_replace(
                    out=work[:], in_to_replace=tv[:, sl], in_values=work[:],
                    imm_value=NEG
                )
            nc.sync.dma_start(topvi[:], tvi[:])

    nc.compile()
    return nc


def _get_program(**kw):
    key = tuple(sorted(kw.items()))
    if key not in _PROGRAMS:
        if kw.pop("sparse", False):
            _PROGRAMS[key] = _build_sparse_program(**kw)
        else:
            _PROGRAMS[key] = _build_program(**kw)
    return _PROGRAMS[key]


def _fp16_split(x):
    hi = x.astype(np.float16)
    lo = (x - hi.astype(np.float32)).astype(np.float16)
    return hi, lo


def _prepare_in_maps(q_vectors, token_ids, vectors, emb2pid, passes=PASSES):
    q = np.ascontiguousarray(np.asarray(q_vectors, dtype=np.float32))
    V = np.asarray(vectors, dtype=np.float32)
    tok = np.asarray(token_ids).astype(np.int64)
    e2p = np.asarray(emb2pid).astype(np.int64)

    qt = np.ascontiguousarray(q.reshape(B * NQ, DIM).T)      # [128, 256]
    if passes == 3:
        qh, ql = _fp16_split(qt)
        qw_np = np.concatenate([qh, ql], axis=1)             # [128, 512]
    else:
        qw_np = qt.astype(np.float16)

    # host-side candidate masks: pids hit per batch, mapped to local doc ids
    pids = e2p[np.clip(tok, 0, N_EMB - 1)]                   # [B, NTOK]
    pids = np.where((tok < 0) | (tok >= N_EMB), -1, pids)
    pids = np.where((pids < 0) | (pids >= N_DOCS), -1, pids)

    in_maps = []
    for c in range(NCORES):
        vs = V[c * SHARD : (c + 1) * SHARD]                  # [625, 128, 128]
        vt = vs.transpose(2, 0, 1).reshape(DIM, SHARD * DOC_LEN)
        vt_p = np.zeros((DIM, COLS), np.float32)
        vt_p[:, : SHARD * DOC_LEN] = vt
        mask = np.full((B, SH_DOCS), NEG, np.float32)
        lo, hi = c * SHARD, (c + 1) * SHARD
        for b in range(B):
            local = pids[b][(pids[b] >= lo) & (pids[b] < hi)] - lo
            mask[b, local] = 0.0
        m = {"qw": qw_np, "maskd": mask}
        if passes == 3:
            m["vh"], m["vl"] = _fp16_split(vt_p)
        else:
            m["vh"] = vt_p.astype(np.float16)
        in_maps.append(m)
    return in_maps


def _prepare_sparse_in_maps(q_vectors, token_ids, vectors, emb2pid):
    """Compute per-batch candidate docs, balance them across cores, and pack
    per-(core, query-half) compacted fp16 V^T layouts + slot masks.
    Returns (in_maps, sels) or None if any core-half overflows S_SLOTS."""
    q = np.ascontiguousarray(np.asarray(q_vectors, dtype=np.float32))
    V = np.asarray(vectors, dtype=np.float32)
    tok = np.asarray(token_ids).astype(np.int64)
    e2p = np.asarray(emb2pid).astype(np.int64)

    qt = np.ascontiguousarray(q.reshape(B * NQ, DIM).T)      # [128, 256]
    qw_np = qt.astype(np.float16)

    pids = e2p[np.clip(tok, 0, N_EMB - 1)]
    pids = np.where((tok < 0) | (tok >= N_EMB), -1, pids)
    pids = np.where((pids < 0) | (pids >= N_DOCS), -1, pids)

    need = np.zeros((B, N_DOCS), bool)
    for b in range(B):
        u = pids[b][pids[b] >= 0]
        need[b, u] = True
    need_h = np.stack([need[:4].any(axis=0), need[4:].any(axis=0)])  # [2, N]

    # balanced doc -> core assignment: round-robin within each need category
    core_of = np.full(N_DOCS, -1, np.int32)
    for m in (need_h[0] & need_h[1], need_h[0] & ~need_h[1],
              ~need_h[0] & need_h[1]):
        idx = np.where(m)[0]
        core_of[idx] = np.arange(len(idx)) % NCORES

    all_sels = []
    max_count = 1
    for c in range(NCORES):
        csel = []
        for h in range(2):
            sel = np.where((core_of == c) & need_h[h])[0]
            if len(sel) > S_SLOTS:
                return None  # overflow: caller falls back to dense
            max_count = max(max_count, len(sel))
            csel.append(sel)
        all_sels.append(csel)
    n_active = -(-max_count // CHUNK2)  # active 32-doc chunks per half
    HC = n_active * CHUNK2 * DOC_LEN

    in_maps = []
    for c in range(NCORES):
        vh_np = np.zeros((DIM, 2 * HC), np.float16)
        mask = np.full((4, 2 * S_SLOTS), NEG, np.float32)
        for h in range(2):
            sel = all_sels[c][h]
            if len(sel):
                vt = V[sel].transpose(2, 0, 1).reshape(DIM, len(sel) * DOC_LEN)
                vh_np[:, h * HC : h * HC + vt.shape[1]] = vt.astype(np.float16)
            for j in range(4):
                mask[j, h * S_SLOTS : h * S_SLOTS + len(sel)] = np.where(
                    need[4 * h + j, sel], 0.0, NEG)
        in_maps.append({"qw": qw_np, "vh": vh_np, "maskd": mask})
    return in_maps, all_sels, n_active


def _merge_sparse(results, sels, n_active, k_val):
    top_scores = np.empty((B, k_val), np.float32)
    top_pids = np.empty((B, k_val), np.int32)
    all_v = [[] for _ in range(B)]
    all_i = [[] for _ in range(B)]
    hp = 4 * n_active  # partitions per half
    for c in range(NCORES):
        tvi = np.asarray(results[c]["topvi"], np.float32)  # [2*hp, 2*TOP_EXPORT]
        tv = tvi[:, :TOP_EXPORT]
        ti = tvi[:, TOP_EXPORT:].copy().view(np.uint32).astype(np.int64)
        for p in range(tvi.shape[0]):
            h, b_loc, ch = p // hp, (p % hp) // n_active, p % n_active
            b = 4 * h + b_loc
            slots = ch * CHUNK2 + ti[p]
            sel = sels[c][h]
            valid = (tv[p] > -1.0e29) & (slots < len(sel))
            all_v[b].append(tv[p][valid])
            all_i[b].append(sel[slots[valid]])
    for b in range(B):
        v = np.concatenate(all_v[b])
        i = np.concatenate(all_i[b])
        order = np.argsort(-v, kind="stable")[:k_val]
        top_scores[b] = v[order]
        top_pids[b] = i[order].astype(np.int32)
    return top_scores, top_pids


def _merge(results, k_val):
    top_scores = np.empty((B, k_val), np.float32)
    top_pids = np.empty((B, k_val), np.int32)
    nchunk = SH_DOCS // CHUNK  # 16
    all_v = [[] for _ in range(B)]
    all_i = [[] for _ in range(B)]
    for c in range(NCORES):
        tvi = np.asarray(results[c]["topvi"], np.float32)
        tv = tvi[:, :CHUNK].reshape(B, nchunk, CHUNK)
        ti = (tvi[:, CHUNK:].copy().view(np.uint32).astype(np.int64)
              .reshape(B, nchunk, CHUNK))
        base = c * SHARD + np.arange(nchunk)[:, None] * CHUNK  # [nchunk, 1]
        for b in range(B):
            all_v[b].append(tv[b].reshape(-1))
            all_i[b].append((ti[b] + base).reshape(-1))
    for b in range(B):
        v = np.concatenate(all_v[b])
        i = np.concatenate(all_i[b])
        valid = v > -1.0e29
        v = v[valid]
        i = i[valid]
        order = np.argsort(-v, kind="stable")[:k_val]
        top_scores[b] = v[order]
        top_pids[b] = i[order].astype(np.int32)
    return top_scores, top_pids


def _run(inputs, trace=False, trace_kwargs=None, program_kwargs=None):
    from concourse.bass_utils import run_bass_kernel_spmd

    pk = dict(program_kwargs or {})
    k_val = int(np.asarray(inputs.get("k", 100)))
    dense = pk.pop("dense", False)
    prep = None
    if not dense and pk.get("passes", PASSES) == 1:
        prep = _prepare_sparse_in_maps(
            inputs["q_vectors"], inputs["token_ids"], inputs["vectors"],
            inputs["emb2pid"])
    if prep is not None:
        in_maps, sels, n_active = prep
        nc = _get_program(sparse=True, n_active=n_active,
                          **{k: v for k, v in pk.items() if k in ("n_a", "n_b")})
        br = run_bass_kernel_spmd(
            nc, in_maps, list(range(NCORES)), trace=trace, **(trace_kwargs or {})
        )
        outs = _merge_sparse(br.results, sels, n_active, k_val)
        return outs, br
    nc = _get_program(**pk)
    in_maps = _prepare_in_maps(
        inputs["q_vectors"], inputs["token_ids"], inputs["vectors"],
        inputs["emb2pid"], passes=pk.get("passes", PASSES)
    )
    br = run_bass_kernel_spmd(
        nc, in_maps, list(range(NCORES)), trace=trace, **(trace_kwargs or {})
    )
    outs = _merge(br.results, k_val)
    return outs, br


def kernel(q_vectors, token_ids, vectors, emb2pid, k=100):
    outs, _ = _run(
        {
            "q_vectors": q_vectors,
            "token_ids": token_ids,
            "vectors": vectors,
            "emb2pid": emb2pid,
            "k": k,
        }
    )
    return outs
